# revision 9
# baseline (speedup 1.0000x reference)
"""Cosine-similarity (2-slot Hungarian-matched) loss on 8 Trainium2 cores.

Math (per sample b, slots i,j in {0,1}):
    cos[i,j] = <pred[b,i]/|pred[b,i]|, gt[b,j]/|gt[b,j]|>
    best = max(cos00+cos11, cos01+cos10)
    loss = mean_b(1 - best/2)

Distribution: pure data parallel — B=32768 is split into 8 shards of 4096.
Each core streams its shard through SBUF in 16 tiles of 256 samples
([128 partitions x 4096 f32] per tensor, 2 samples per partition, 2 MiB
per DMA, 8-deep buffer pool).  Engine balance (measured: ACT norm op
1426ns, DVE dot op 1305ns, DMA span ~181us/core at ~370GB/s): ScalarE
computes 122 of the 128 squared-norm ops with fused Square+accumulate;
VectorE computes all 128 cross dot products with fused STT+accumulate
plus the first 6 ng1 norm columns (front-loaded so the transient lands
during the DMA ramp).  A tiny epilogue normalizes
(cos = c * exp(-0.5*ln(np*ng))), picks max(id, swap) and reduces to a
[128,1] per-core partial sum of best_sum.  The host adds the 8*128
partials and finishes 1 - total/(2B).
"""

import sys

import numpy as np

sys.path.insert(0, "/opt/trn_rl_repo")

import concourse.bacc as bacc
import concourse.bass as bass
import concourse.mybir as mybir
import concourse.tile as tile
from concourse.bass_utils import run_bass_kernel_spmd

B, S, D = 32768, 2, 1024
N_CORES = 8
B_C = B // N_CORES          # samples per core
NPART = 128
TILE_S = 256                # samples per SBUF tile (2 per partition)
NSUB = TILE_S // NPART      # samples per partition
NT = B_C // TILE_S          # tiles per core
NCOL = NT * NSUB            # stat columns per partition
F32 = mybir.dt.float32
AF = mybir.ActivationFunctionType
ALU = mybir.AluOpType


def build_nc(b_c=B_C, tile_s=TILE_S, input_bufs=6, n_dve_norms=6):
    """Two-engine balanced split (measured: ACT op 1426ns, DVE op 1305ns
    on [128,1024]; DMA span ~181us):
      ACT: all squared-norm accumulates except the first `n_dve_norms`
           ng1 columns (st_n regions [np0|np1|ng0|ng1], ng1 cols
           0..n_dve_norms-1 left unwritten)
      DVE: all 128 cross dots (st_c regions c00,c01,c10,c11) plus the
           first n_dve_norms ng1 norms (st_c region 4, front-loaded so
           the transient DVE overload lands during the DMA ramp)
    """
    nsub = tile_s // NPART
    nt = b_c // tile_s
    ncol = nt * nsub

    nc = bacc.Bacc(trn_type="TRN2")
    pred_h = nc.declare_dram_parameter("pred", [b_c, S, D], F32, isOutput=False)
    gt_h = nc.declare_dram_parameter("gt", [b_c, S, D], F32, isOutput=False)
    out_h = nc.declare_dram_parameter("out", [NPART, 1], F32, isOutput=True)

    # tile i, partition p holds samples (i*tile_s + p*nsub + j), j<nsub, each
    # a contiguous s*d run -> per-partition rows are nsub*S*D contiguous f32.
    pred_ap = pred_h[:].rearrange("(t p n) s d -> t p (n s d)", p=NPART, n=nsub)
    gt_ap = gt_h[:].rearrange("(t p n) s d -> t p (n s d)", p=NPART, n=nsub)

    with tile.TileContext(nc) as tc:
        with (
            tc.tile_pool(name="pin", bufs=input_bufs) as pin,
            tc.tile_pool(name="stats", bufs=1) as stats,
            tc.tile_pool(name="scratch", bufs=1) as scratch,
            tc.tile_pool(name="epi", bufs=1) as epi,
        ):
            # norms (ACT): st_n regions [np0 | np1 | ng0 | ng1]
            # crosses (DVE): st_c regions [c00|c01|c10|c11|ng1_dve]
            st_n = stats.tile([NPART, 4 * ncol], F32, tag="st_n", name="st_n")
            st_c = stats.tile([NPART, 5 * ncol], F32, tag="st_c", name="st_c")

            # Pre-load ACT table set 6 (natural_log_exp_and_others): it holds
            # square+ln+exp, so the whole kernel runs off one table load
            # instead of the 0->5->0 bounce the greedy inserter would emit.
            nc.scalar.add_instruction(
                mybir.InstLoadActFuncSet(
                    name=nc.get_next_instruction_name(),
                    act_func_set_id=6,
                    ins=[],
                    outs=[],
                )
            )
            scr_a = scratch.tile([NPART, D], F32, tag="scr_a", name="scr_a")
            scr_v = scratch.tile([NPART, D], F32, tag="scr_v", name="scr_v")

            for i in range(nt):
                p_t = pin.tile([NPART, nsub * S * D], F32, tag="P", name="P")
                g_t = pin.tile([NPART, nsub * S * D], F32, tag="G", name="G")
                nc.sync.dma_start(out=p_t[:], in_=pred_ap[i])
                nc.sync.dma_start(out=g_t[:], in_=gt_ap[i])
                for j in range(nsub):
                    col = i * nsub + j
                    p0 = p_t[:, (j * S + 0) * D:(j * S + 1) * D]
                    p1 = p_t[:, (j * S + 1) * D:(j * S + 2) * D]
                    g0 = g_t[:, (j * S + 0) * D:(j * S + 1) * D]
                    g1 = g_t[:, (j * S + 1) * D:(j * S + 2) * D]
                    for reg, src in ((0, p0), (1, p1), (2, g0), (3, g1)):
                        if reg == 3 and col < n_dve_norms:
                            nc.vector.scalar_tensor_tensor(
                                out=scr_v[:], in0=src, scalar=1.0, in1=src,
                                op0=ALU.mult, op1=ALU.mult,
                                accum_out=st_c[:, 4 * ncol + col:4 * ncol + col + 1],
                            )
                            continue
                        c0 = reg * ncol + col
                        nc.scalar.activation(
                            scr_a[:], src, AF.Square,
                            accum_out=st_n[:, c0:c0 + 1],
                        )
                    for reg, a, b in (
                        (0, p0, g0), (1, p0, g1),
                        (2, p1, g0), (3, p1, g1),
                    ):
                        c0 = reg * ncol + col
                        nc.vector.scalar_tensor_tensor(
                            out=scr_v[:], in0=a, scalar=1.0, in1=b,
                            op0=ALU.mult, op1=ALU.mult,
                            accum_out=st_c[:, c0:c0 + 1],
                        )

            # merge the DVE-computed ng1 columns into one contiguous row
            ng1_m = epi.tile([NPART, ncol], F32, tag="ng1_m", name="ng1_m")
            if n_dve_norms:
                nc.vector.tensor_copy(
                    out=ng1_m[:, 0:n_dve_norms],
                    in_=st_c[:, 4 * ncol:4 * ncol + n_dve_norms],
                )
            nc.vector.tensor_copy(
                out=ng1_m[:, n_dve_norms:ncol],
                in_=st_n[:, 3 * ncol + n_dve_norms:4 * ncol],
            )

            # epilogue: wide fused ops over the contiguous stat regions.
            # t_all regions (i,j) follow st_c's (c00, c01, c10, c11) order.
            t_all = epi.tile([NPART, 4 * ncol], F32, tag="t_all", name="t_all")
            for idx, (i_, j_) in enumerate(((0, 0), (0, 1), (1, 0), (1, 1))):
                ng = st_n[:, 2 * ncol:3 * ncol] if j_ == 0 else ng1_m[:]
                nc.vector.tensor_mul(
                    t_all[:, idx * ncol:(idx + 1) * ncol],
                    st_n[:, i_ * ncol:(i_ + 1) * ncol],
                    ng,
                )
            # rsqrt via exp(-0.5*ln(t)); in-place wide ACT ops
            nc.scalar.activation(t_all[:], t_all[:], AF.Ln)
            nc.scalar.activation(t_all[:], t_all[:], AF.Exp, scale=-0.5)
            cos_all = epi.tile([NPART, 4 * ncol], F32, tag="cos_all", name="cos_all")
            nc.vector.tensor_mul(cos_all[:], st_c[:, 0:4 * ncol], t_all[:])
            s_id = epi.tile([NPART, ncol], F32, tag="s_id", name="s_id")
            s_sw = epi.tile([NPART, ncol], F32, tag="s_sw", name="s_sw")
            nc.vector.tensor_add(s_id[:], cos_all[:, 0:ncol], cos_all[:, 3 * ncol:4 * ncol])
            nc.vector.tensor_add(s_sw[:], cos_all[:, ncol:2 * ncol], cos_all[:, 2 * ncol:3 * ncol])
            best = epi.tile([NPART, ncol], F32, tag="best", name="best")
            partial = epi.tile([NPART, 1], F32, tag="partial", name="partial")
            nc.vector.tensor_max(best[:], s_id[:], s_sw[:])
            nc.vector.reduce_sum(partial[:], best[:], axis=mybir.AxisListType.X)
            nc.sync.dma_start(out=out_h[:], in_=partial[:])
    nc.finalize()
    return nc


_CACHE = {}


def _get_nc():
    if "nc" not in _CACHE:
        _CACHE["nc"] = build_nc()
    return _CACHE["nc"]


def run_spmd(pred, gt, **kwargs):
    """Run the SPMD kernel; returns (BassKernelResults, per-core partials)."""
    pred = np.ascontiguousarray(np.asarray(pred), dtype=np.float32)
    gt = np.ascontiguousarray(np.asarray(gt), dtype=np.float32)
    assert pred.shape == (B, S, D) and gt.shape == (B, S, D)
    nc = _get_nc()
    in_maps = [
        {"pred": pred[c * B_C:(c + 1) * B_C], "gt": gt[c * B_C:(c + 1) * B_C]}
        for c in range(N_CORES)
    ]
    res = run_bass_kernel_spmd(nc, in_maps, list(range(N_CORES)), **kwargs)
    return res


def kernel(pred, gt):
    res = run_spmd(pred, gt)
    total = sum(
        float(np.sum(r["out"], dtype=np.float64)) for r in res.results
    )
    loss = 1.0 - total / (2.0 * B)
    return np.array(loss, dtype=np.float32)



# revision 11
# speedup vs baseline: 1.0297x; 1.0297x over previous
"""Cosine-similarity (2-slot Hungarian-matched) loss on 8 Trainium2 cores.

Math (per sample b, slots i,j in {0,1}):
    cos[i,j] = <pred[b,i]/|pred[b,i]|, gt[b,j]/|gt[b,j]|>
    best = max(cos00+cos11, cos01+cos10)
    loss = mean_b(1 - best/2)

Distribution: pure data parallel — B=32768 is split into 8 shards of 4096.
Each core streams its shard through SBUF in 16 tiles of 256 samples
([128 partitions x 4096 f32] per tensor, 2 samples per partition, 2 MiB
per DMA, 8-deep buffer pool).  Engine balance (measured: ACT norm op
1426ns, DVE dot op 1305ns, DMA span ~181us/core at ~370GB/s): ScalarE
computes 122 of the 128 squared-norm ops with fused Square+accumulate;
VectorE computes all 128 cross dot products with fused STT+accumulate
plus the first 6 ng1 norm columns (front-loaded so the transient lands
during the DMA ramp).  A tiny epilogue normalizes
(cos = c * exp(-0.5*ln(np*ng))), picks max(id, swap) and reduces to a
[128,1] per-core partial sum of best_sum.  The host adds the 8*128
partials and finishes 1 - total/(2B).
"""

import sys

import numpy as np

sys.path.insert(0, "/opt/trn_rl_repo")

import concourse.bacc as bacc
import concourse.bass as bass
import concourse.mybir as mybir
import concourse.tile as tile
from concourse.bass_utils import run_bass_kernel_spmd

B, S, D = 32768, 2, 1024
N_CORES = 8
B_C = B // N_CORES          # samples per core
NPART = 128
TILE_S = 128                # samples per SBUF tile (1 per partition;
                            # 8KiB DMA rows beat 16KiB: 370 vs 336 GB/s)
NSUB = TILE_S // NPART      # samples per partition
NT = B_C // TILE_S          # tiles per core
NCOL = NT * NSUB            # stat columns per partition
F32 = mybir.dt.float32
AF = mybir.ActivationFunctionType
ALU = mybir.AluOpType


def build_nc(b_c=B_C, tile_s=TILE_S, input_bufs=10, n_dve_norms=6):
    """Two-engine balanced split (measured: ACT op 1426ns, DVE op 1305ns
    on [128,1024]; DMA span ~181us):
      ACT: all squared-norm accumulates except the first `n_dve_norms`
           ng1 columns (st_n regions [np0|np1|ng0|ng1], ng1 cols
           0..n_dve_norms-1 left unwritten)
      DVE: all 128 cross dots (st_c regions c00,c01,c10,c11) plus the
           first n_dve_norms ng1 norms (st_c region 4, front-loaded so
           the transient DVE overload lands during the DMA ramp)
    """
    nsub = tile_s // NPART
    nt = b_c // tile_s
    ncol = nt * nsub

    nc = bacc.Bacc(trn_type="TRN2")
    pred_h = nc.declare_dram_parameter("pred", [b_c, S, D], F32, isOutput=False)
    gt_h = nc.declare_dram_parameter("gt", [b_c, S, D], F32, isOutput=False)
    out_h = nc.declare_dram_parameter("out", [NPART, 1], F32, isOutput=True)

    # tile i, partition p holds samples (i*tile_s + p*nsub + j), j<nsub, each
    # a contiguous s*d run -> per-partition rows are nsub*S*D contiguous f32.
    pred_ap = pred_h[:].rearrange("(t p n) s d -> t p (n s d)", p=NPART, n=nsub)
    gt_ap = gt_h[:].rearrange("(t p n) s d -> t p (n s d)", p=NPART, n=nsub)

    with tile.TileContext(nc) as tc:
        with (
            tc.tile_pool(name="pin", bufs=input_bufs) as pin,
            tc.tile_pool(name="stats", bufs=1) as stats,
            tc.tile_pool(name="scratch", bufs=1) as scratch,
            tc.tile_pool(name="epi", bufs=1) as epi,
        ):
            # norms (ACT): st_n regions [np0 | np1 | ng0 | ng1]
            # crosses (DVE): st_c regions [c00|c01|c10|c11|ng1_dve]
            st_n = stats.tile([NPART, 4 * ncol], F32, tag="st_n", name="st_n")
            st_c = stats.tile([NPART, 5 * ncol], F32, tag="st_c", name="st_c")

            # Pre-load ACT table set 6 (natural_log_exp_and_others): it holds
            # square+ln+exp, so the whole kernel runs off one table load
            # instead of the 0->5->0 bounce the greedy inserter would emit.
            nc.scalar.add_instruction(
                mybir.InstLoadActFuncSet(
                    name=nc.get_next_instruction_name(),
                    act_func_set_id=6,
                    ins=[],
                    outs=[],
                )
            )
            scr_a = scratch.tile([NPART, D], F32, tag="scr_a", name="scr_a")
            scr_v = scratch.tile([NPART, D], F32, tag="scr_v", name="scr_v")

            for i in range(nt):
                p_t = pin.tile([NPART, nsub * S * D], F32, tag="P", name="P")
                g_t = pin.tile([NPART, nsub * S * D], F32, tag="G", name="G")
                nc.sync.dma_start(out=p_t[:], in_=pred_ap[i])
                nc.sync.dma_start(out=g_t[:], in_=gt_ap[i])
                for j in range(nsub):
                    col = i * nsub + j
                    p0 = p_t[:, (j * S + 0) * D:(j * S + 1) * D]
                    p1 = p_t[:, (j * S + 1) * D:(j * S + 2) * D]
                    g0 = g_t[:, (j * S + 0) * D:(j * S + 1) * D]
                    g1 = g_t[:, (j * S + 1) * D:(j * S + 2) * D]
                    for reg, src in ((0, p0), (1, p1), (2, g0), (3, g1)):
                        if reg == 3 and col < n_dve_norms:
                            nc.vector.scalar_tensor_tensor(
                                out=scr_v[:], in0=src, scalar=1.0, in1=src,
                                op0=ALU.mult, op1=ALU.mult,
                                accum_out=st_c[:, 4 * ncol + col:4 * ncol + col + 1],
                            )
                            continue
                        c0 = reg * ncol + col
                        nc.scalar.activation(
                            scr_a[:], src, AF.Square,
                            accum_out=st_n[:, c0:c0 + 1],
                        )
                    for reg, a, b in (
                        (0, p0, g0), (1, p0, g1),
                        (2, p1, g0), (3, p1, g1),
                    ):
                        c0 = reg * ncol + col
                        nc.vector.scalar_tensor_tensor(
                            out=scr_v[:], in0=a, scalar=1.0, in1=b,
                            op0=ALU.mult, op1=ALU.mult,
                            accum_out=st_c[:, c0:c0 + 1],
                        )

            # merge the DVE-computed ng1 columns into one contiguous row
            ng1_m = epi.tile([NPART, ncol], F32, tag="ng1_m", name="ng1_m")
            if n_dve_norms:
                nc.vector.tensor_copy(
                    out=ng1_m[:, 0:n_dve_norms],
                    in_=st_c[:, 4 * ncol:4 * ncol + n_dve_norms],
                )
            nc.vector.tensor_copy(
                out=ng1_m[:, n_dve_norms:ncol],
                in_=st_n[:, 3 * ncol + n_dve_norms:4 * ncol],
            )

            # epilogue: wide fused ops over the contiguous stat regions.
            # t_all regions (i,j) follow st_c's (c00, c01, c10, c11) order.
            t_all = epi.tile([NPART, 4 * ncol], F32, tag="t_all", name="t_all")
            for idx, (i_, j_) in enumerate(((0, 0), (0, 1), (1, 0), (1, 1))):
                ng = st_n[:, 2 * ncol:3 * ncol] if j_ == 0 else ng1_m[:]
                nc.vector.tensor_mul(
                    t_all[:, idx * ncol:(idx + 1) * ncol],
                    st_n[:, i_ * ncol:(i_ + 1) * ncol],
                    ng,
                )
            # rsqrt via exp(-0.5*ln(t)); in-place wide ACT ops
            nc.scalar.activation(t_all[:], t_all[:], AF.Ln)
            nc.scalar.activation(t_all[:], t_all[:], AF.Exp, scale=-0.5)
            cos_all = epi.tile([NPART, 4 * ncol], F32, tag="cos_all", name="cos_all")
            nc.vector.tensor_mul(cos_all[:], st_c[:, 0:4 * ncol], t_all[:])
            s_id = epi.tile([NPART, ncol], F32, tag="s_id", name="s_id")
            s_sw = epi.tile([NPART, ncol], F32, tag="s_sw", name="s_sw")
            nc.vector.tensor_add(s_id[:], cos_all[:, 0:ncol], cos_all[:, 3 * ncol:4 * ncol])
            nc.vector.tensor_add(s_sw[:], cos_all[:, ncol:2 * ncol], cos_all[:, 2 * ncol:3 * ncol])
            best = epi.tile([NPART, ncol], F32, tag="best", name="best")
            partial = epi.tile([NPART, 1], F32, tag="partial", name="partial")
            nc.vector.tensor_max(best[:], s_id[:], s_sw[:])
            nc.vector.reduce_sum(partial[:], best[:], axis=mybir.AxisListType.X)
            nc.sync.dma_start(out=out_h[:], in_=partial[:])
    nc.finalize()
    return nc


_CACHE = {}


def _get_nc():
    if "nc" not in _CACHE:
        _CACHE["nc"] = build_nc()
    return _CACHE["nc"]


def run_spmd(pred, gt, **kwargs):
    """Run the SPMD kernel; returns (BassKernelResults, per-core partials)."""
    pred = np.ascontiguousarray(np.asarray(pred), dtype=np.float32)
    gt = np.ascontiguousarray(np.asarray(gt), dtype=np.float32)
    assert pred.shape == (B, S, D) and gt.shape == (B, S, D)
    nc = _get_nc()
    in_maps = [
        {"pred": pred[c * B_C:(c + 1) * B_C], "gt": gt[c * B_C:(c + 1) * B_C]}
        for c in range(N_CORES)
    ]
    res = run_bass_kernel_spmd(nc, in_maps, list(range(N_CORES)), **kwargs)
    return res


def kernel(pred, gt):
    res = run_spmd(pred, gt)
    total = sum(
        float(np.sum(r["out"], dtype=np.float64)) for r in res.results
    )
    loss = 1.0 - total / (2.0 * B)
    return np.array(loss, dtype=np.float32)

